# revision 41
# baseline (speedup 1.0000x reference)
"""Causal self-attention Trainium2 kernel (v9, 258.2us vs 320.4 baseline).

Sharding: 8 cores = (4 batches) x (2 head-groups of 8 heads).
Each core: projections for its 512 channels, causal attention for its 8
heads over its batch, partial out-projection over its 512 channels.
Host: sums the two partials per batch and adds bo + bv@wo.

Design (measured on HW):
- Projections are FUSED into the attention pipeline: q/k/v chains for
  later blocks pop into the attention PE windows (proj_fill + GATE), so
  the engine-heavy small-ic pairs overlap projection matmuls.
- Scores run row-tiled: head-even (SBUF parts 0:64) on PE tile (0,0)
  and head-odd (64:128) on tile (64,0) concurrently (~142.6ns/MM-pair
  vs 250 serial).  q/k projections are fp8 DoubleRow.
- No mask matmuls: diagonal-tile exp runs on DVE as Schraudolph
  fast-exp (i16 = rint(score*A + add) bitcast bf16, ~2% rel err) whose
  per-element add-constant tensor bm carries the causal mask
  (B - 14000 on dead elements -> weight ~2^-109).  Off-diagonal exp
  slabs split ACT/DVE per DVE_OFF to balance the engines.
- Softmax denominator: ones-columns in vx ride the attnV matmul; the
  reciprocal is exp(-ln(den)) -- two ACT spline passes (ln+exp live in
  one table set), ~1e-6 rel; one DVE multiply normalizes.  v-bias is
  folded into the host output bias (attn(v+bv) = attn(v)+bv).
- PE mode windows per head-pair: [scores 64-mode] [prev pair's attnV +
  out-proj fills + proj fills, 128-mode]; switches cost ~216ns.
- PSUM: pj 2 + sc 2x2 + at/op 2 banks; all projections drain by the
  start of ic=2, where the pj pool closes and a third sc buffer (sc2)
  opens for the ic=2/3 pairs (deeper score-exp rotation).
- x streams in as per-block tiles (xt8 per 512, xts per 128 tokens);
  input DMAs are priority-serialized on the sync queue so the first
  q-chain starts ~12us in.
"""

from contextlib import ExitStack

import ml_dtypes
import numpy as np

import concourse.bass as bass
import concourse.mybir as mybir
import concourse.tile as tile

P = 128
C = 1024  # d_model
CL = 512  # local channels (8 heads * 64)
D = 64  # head dim
NH = 8  # local heads
FC = C // P  # 8 f-chunks
OC = CL // P  # 4 o-chunks
F32 = mybir.dt.float32
BF16 = mybir.dt.bfloat16
F8 = mybir.dt.float8e4
I16 = mybir.dt.int16
I32 = mybir.dt.int32
AF = mybir.ActivationFunctionType
ALU = mybir.AluOpType
DR = mybir.MatmulPerfMode.DoubleRow

# Schraudolph fast-exp constants: exp(s*0.125) ~ bf16(bitcast(rint(s*A+B)))
EXP_A = 0.125 * 128.0 / float(np.log(2.0))
EXP_B = 127.0 * 128.0 - 5.0
MASK_SUB = 14000.0  # bm = EXP_B - MASK_SUB on masked elements -> weight ~2^-109
RECIP_MAGIC = 0x7EF30000  # 1/x seed: bitcast(MAGIC - bits(x)), NR1 -> 2.6e-3
CH = 8  # score jt-tiles per PE mode window
# off-diagonal exp slabs sent to DVE per pair, by ic (diag slabs are always
# DVE: their Schraudolph add-constant carries the causal mask for free)
DVE_OFF = {0: 0, 1: 2, 2: 3, 3: 4}
V_BIAS_DVE = 1  # debug: 0 -> plain scalar copy (wrong bias, compile test)


def _emit(nc, tc, ctx, T):
    NT = T // P  # 16 token chunks
    T4 = T // 512  # 4 ic-blocks

    xTd = nc.dram_tensor("xT", [P, FC, T], BF16, kind="ExternalInput")
    xT8d = nc.dram_tensor("xT8", [P, FC, T], F8, kind="ExternalInput")
    wq8d = nc.dram_tensor("wq8", [P, FC, CL], F8, kind="ExternalInput")
    wk8d = nc.dram_tensor("wk8", [P, FC, CL], F8, kind="ExternalInput")
    wvd = nc.dram_tensor("wv", [P, FC, CL], BF16, kind="ExternalInput")
    wod = nc.dram_tensor("wo", [P, OC, C], BF16, kind="ExternalInput")
    bqd = nc.dram_tensor("bq", [CL], F32, kind="ExternalInput")
    bkd = nc.dram_tensor("bk", [CL], F32, kind="ExternalInput")
    bmd = nc.dram_tensor("bm", [P, 4, 512], I16, kind="ExternalInput")
    outp = nc.dram_tensor("outp", [T, C], F32, kind="ExternalOutput")

    const = ctx.enter_context(tc.tile_pool(name="const", bufs=1))
    bq_sb = const.tile([P, OC], F32)
    nc.sync.dma_start(bq_sb[:], bqd.rearrange("(oc p) -> p oc", p=P))
    bk_sb = const.tile([P, OC], F32)
    nc.sync.dma_start(bk_sb[:], bkd.rearrange("(oc p) -> p oc", p=P))
    bm_sb = const.tile([P, 4, 512], I16)

    main = ctx.enter_context(tc.tile_pool(name="main", bufs=1))
    qT = main.tile([P, OC, T], BF16)
    kT = main.tile([P, OC, T], BF16)
    vx = main.tile([P, NT, NH, P], BF16)
    wo_sb = main.tile([P, OC, C], BF16)

    # ---- fused projections + attention + out-projection ----
    with (
        tc.tile_pool(name="att_pool", bufs=2) as att_pool,
        tc.tile_pool(name="ex_pool", bufs=2) as ex_pool,
        tc.tile_pool(name="rc_pool", bufs=2) as rc_pool,
        tc.tile_pool(name="ln_pool", bufs=1) as ln_pool,
        tc.tile_pool(name="ob_pool", bufs=2) as ob_pool,
        tc.tile_pool(name="sc_ps", bufs=2, space="PSUM") as sc_ps,
        tc.tile_pool(name="atop_ps", bufs=2, space="PSUM") as atop_ps,
    ):
        vstack = ExitStack()
        vpool = vstack.enter_context(tc.tile_pool(name="vpool", bufs=1))
        qkstack = ExitStack()
        qkpool = qkstack.enter_context(tc.tile_pool(name="qkpool", bufs=1))
        pjstack = ExitStack()
        pj_ps = pjstack.enter_context(
            tc.tile_pool(name="pj_ps", bufs=2, space="PSUM")
        )
        sc2stack = ExitStack()
        sc2_box = []

        wq8_sb = qkpool.tile([P, FC, CL], F8)
        wk8_sb = qkpool.tile([P, FC, CL], F8)
        wv_sb = vpool.tile([P, FC, CL], BF16)

        # x arrives as streamed per-block tiles: xt8 per 512-token block
        # (q/k), xts per 128-token chunk (v)
        xt8_tiles = {}

        def fetch_xt8(tt):
            if tt in xt8_tiles or tt >= T4:
                return
            t = qkpool.tile([P, FC, 512], F8, tag="xt8", name=f"xt8_{tt}", bufs=2)
            nc.sync.dma_start(t[:], xT8d[:, :, tt * 512 : (tt + 1) * 512])
            xt8_tiles[tt] = t

        xts_tiles = {}

        def fetch_xts(s):
            if s in xts_tiles or s >= NT:
                return
            t = vpool.tile([P, FC, P], BF16, tag="xts", name=f"xts_{s}", bufs=4)
            nc.sync.dma_start(t[:], xTd[:, :, s * P : (s + 1) * P])
            xts_tiles[s] = t

        # load order: the oc=0 slice of wq8 and the first half of xt8_0
        # land first so the first q chain starts ~9us in
        nc.scalar.dma_start(wq8_sb[:, :, 0:P], wq8d[:, :, 0:P])
        t0_ = qkpool.tile([P, FC, 512], F8, tag="xt8", name="xt8_0", bufs=2)
        nc.sync.dma_start(t0_[:, 0:4, :], xT8d[:, 0:4, 0:512])
        nc.sync.dma_start(t0_[:, 4:8, :], xT8d[:, 4:8, 0:512])
        xt8_tiles[0] = t0_
        nc.scalar.dma_start(wq8_sb[:, :, P:CL], wq8d[:, :, P:CL])
        nc.scalar.dma_start(wk8_sb[:], wk8d[:])

        # -- projection unit closures (one PE chain + one evac each) --
        def qk_unit(w8, b_sb, dT, oc, tt):
            def emit():
                ps = pj_ps.tile(
                    [P, 512], F32, tag="pj", name=f"pj{oc}_{tt}_{dT is qT}"
                )
                fetch_xt8(tt + 1)
                xt8 = xt8_tiles[tt]
                for g in range(4):
                    nc.tensor.matmul(
                        ps[:],
                        w8[:, 2 * g : 2 * g + 2, oc * P : (oc + 1) * P],
                        xt8[:, 2 * g : 2 * g + 2, :],
                        start=(g == 0),
                        stop=(g == 3),
                        perf_mode=DR,
                    )
                nc.scalar.activation(
                    dT[:, oc, tt * 512 : (tt + 1) * 512],
                    ps[:],
                    AF.Identity,
                    bias=b_sb[:, oc : oc + 1],
                    scale=0.03125,
                )

            return emit

        def v_unit(s):
            def emit():
                ps = pj_ps.tile([P, 512], F32, tag="pj", name=f"pjv{s}")
                fetch_xts(s + 2)
                xts = xts_tiles[s]
                for fc in range(FC):
                    nc.tensor.matmul(
                        ps[:],
                        xts[:, fc, :],
                        wv_sb[:, fc, :],
                        start=(fc == 0),
                        stop=(fc == FC - 1),
                    )
                # bv is folded into the host-side output bias (bv @ wo)
                nc.scalar.copy(
                    vx[:, s, :, 0:D], ps[:].rearrange("p (h d) -> p h d", d=D)
                )

            return emit

        # block 0 of q/k and the first v chunks run up front; the rest
        # interleaves into the attention windows (proj_fill).  All q chains
        # first (only wq8 + xt8_0 needed), remaining loads issue behind them.
        for oc in range(OC):
            qk_unit(wq8_sb, bq_sb, qT, oc, 0)()
        fetch_xt8(1)
        fetch_xts(0)
        fetch_xts(1)
        nc.vector.memset(vx[:, :, :, D:P], 1.0)
        nc.sync.dma_start(wv_sb[:], wvd[:])
        nc.sync.dma_start(bm_sb[:], bmd[:])
        nc.sync.dma_start(wo_sb[:], wod[:])
        for oc in range(OC):
            qk_unit(wk8_sb, bk_sb, kT, oc, 0)()
        for s in range(4):
            v_unit(s)()
        proj_fill = []
        for tt in (1, 2, 3):
            for oc in range(OC):
                proj_fill.append(qk_unit(wq8_sb, bq_sb, qT, oc, tt))
                proj_fill.append(qk_unit(wk8_sb, bk_sb, kT, oc, tt))
            for s in range(4 * tt, 4 * tt + 4):
                proj_fill.append(v_unit(s))
        GATE = {0: 0, 1: 12, 2: 36, 3: 36}
        popped = [0]

        def pop_proj(n):
            while n > 0 and proj_fill:
                proj_fill.pop(0)()
                popped[0] += 1
                n -= 1

        pairs = [(ic, oc) for ic in (0, 1, 2, 3) for oc in range(OC)]
        att_tiles = {}

        def att_of(ic):
            if ic not in att_tiles:
                att_tiles[ic] = att_pool.tile(
                    [P, OC, 512], BF16, tag="attT", name=f"attT{ic}"
                )
            return att_tiles[ic]

        def make_pair(ic, oc):
            njt = 4 * ic + 4
            ex = ex_pool.tile(
                [P, njt, 2, 512], BF16, tag="ex", name=f"ex{ic}_{oc}"
            )
            at_box = [None, None]

            def sc_jt(jt):
                def emit(jt=jt):
                    pool = sc2_box[0] if (sc2_box and jt % 3 == 2) else sc_ps
                    sc = pool.tile(
                        [P, 2, 512], F32, tag="sc", name=f"sc{ic}_{oc}_{jt}"
                    )
                    d = jt - 4 * ic
                    lo = d * 128 if (d > 0 and ic > 0) else 0
                    q0 = ic * 512
                    for hh in (0, 1):
                        base = hh * D
                        nc.tensor.matmul(
                            sc[:, hh, lo:512],
                            kT[base : base + D, oc, jt * P : (jt + 1) * P],
                            qT[base : base + D, oc, q0 + lo : q0 + 512],
                            start=True,
                            stop=True,
                        )
                    if d >= 0:
                        # diagonal tile: DVE fast-exp; the add-constant tensor
                        # bm carries the causal mask (B - 14000 on dead elems)
                        bmv = bm_sb[:, d, lo:512].copy()
                        bmv.ap.insert(1, (0, 2))
                        nc.vector.scalar_tensor_tensor(
                            ex[:, jt, :, lo:512].bitcast(I16),
                            sc[:, :, lo:512],
                            EXP_A,
                            bmv,
                            ALU.mult,
                            ALU.add,
                        )
                    elif jt < DVE_OFF.get(ic, 0):
                        nc.vector.tensor_scalar(
                            ex[:, jt, :, lo:512].bitcast(I16),
                            sc[:, :, lo:512],
                            EXP_A,
                            EXP_B,
                            ALU.mult,
                            ALU.add,
                        )
                    else:
                        nc.scalar.activation(
                            ex[:, jt, :, lo:512],
                            sc[:, :, lo:512],
                            AF.Exp,
                            scale=0.125,
                        )

                return emit

            def at_jt(jt):
                def emit(jt=jt):
                    if jt == 0:
                        at_box[0] = atop_ps.tile(
                            [P, 512], F32, tag="at", name=f"at{ic}_{oc}_e"
                        )
                        at_box[1] = atop_ps.tile(
                            [P, 512], F32, tag="at", name=f"at{ic}_{oc}_o"
                        )
                    d = jt - 4 * ic
                    alo = d * 128 if d > 0 else 0
                    for hh in (0, 1):
                        nc.tensor.matmul(
                            at_box[hh][:, alo:512],
                            vx[:, jt, 2 * oc + hh, :],
                            ex[:, jt, hh, alo:512],
                            start=(jt == 0),
                            stop=(jt == njt - 1),
                            skip_group_check=True,
                        )

                return emit

            def norm():
                # 1/den via ACT spline passes exp(-ln(den)) (~1e-6 rel),
                # then one DVE multiply
                attn = att_of(ic)
                for hh in (0, 1):
                    base = hh * D
                    lnt = ln_pool.tile(
                        [D, 512], F32, tag="ln", name=f"ln{ic}_{oc}_{hh}"
                    )
                    rc = rc_pool.tile(
                        [D, 512], F32, tag="rc", name=f"rc{ic}_{oc}_{hh}"
                    )
                    nc.scalar.activation(lnt[:], at_box[hh][D:P, :], AF.Ln)
                    nc.scalar.activation(rc[:], lnt[:], AF.Exp, scale=-1.0)
                    nc.vector.tensor_mul(
                        attn[base : base + D, oc, :],
                        at_box[hh][0:D, :],
                        rc[:],
                    )

            return {
                "ic": ic,
                "sc": [sc_jt(j) for j in range(njt)],
                "at": [at_jt(j) for j in range(njt)],
                "norm": norm,
            }

        def op_emitters(ic):
            attn = att_of(ic)
            outs = []
            for s in range(4):
                for ch in range(2):

                    def emit(s=s, ch=ch, ic=ic, attn=attn):
                        s16 = ic * 4 + s
                        ps = atop_ps.tile(
                            [P, 512], F32, tag="at", name=f"op{ic}_{s}_{ch}"
                        )
                        for oc in range(OC):
                            nc.tensor.matmul(
                                ps[:],
                                attn[:, oc, s * P : (s + 1) * P],
                                wo_sb[:, oc, ch * 512 : (ch + 1) * 512],
                                start=(oc == 0),
                                stop=(oc == OC - 1),
                                skip_group_check=True,
                            )
                        ob = ob_pool.tile(
                            [P, 512], F32, tag="ob", name=f"ob{ic}_{s}_{ch}"
                        )
                        nc.vector.tensor_copy(ob[:], ps[:])
                        eng = nc.gpsimd if (s + ch) % 2 == 0 else nc.sync
                        eng.dma_start(
                            outp[s16 * P : (s16 + 1) * P, ch * 512 : (ch + 1) * 512],
                            ob[:],
                        )

                    outs.append(emit)
            return outs

        prev = None
        pending_ops = []
        normed = {i: 0 for i in range(T4)}

        for ic, oc in pairs:
            if oc == 0:
                pop_proj(GATE[ic] - popped[0])
                if ic == 2:
                    pjstack.close()
                    qkstack.close()
                    vstack.close()
                    sc2_box.append(
                        sc2stack.enter_context(
                            tc.tile_pool(name="sc2_ps", bufs=1, space="PSUM")
                        )
                    )
            cur = make_pair(ic, oc)
            njt = len(cur["sc"])
            if prev is None:
                for e in cur["sc"]:
                    e()
            else:
                pat = prev["at"]
                pn = len(pat)
                nw = max((njt + CH - 1) // CH, 1)
                for w in range(nw):
                    for e in cur["sc"][w * CH : (w + 1) * CH]:
                        e()
                    # 128-mode window: prev attnV share + proj fills + op fills
                    a0 = pn * w // nw
                    a1 = pn * (w + 1) // nw
                    for e in pat[a0:a1]:
                        e()
                    if w == nw - 1:
                        prev["norm"]()
                        normed[prev["ic"]] += 1
                        if normed[prev["ic"]] == OC:
                            pending_ops.extend(op_emitters(prev["ic"]))
                    pop_proj(5 if ic <= 1 else 3)
                    for _ in range(3):
                        if pending_ops:
                            pending_ops.pop(0)()
            prev = cur

        pop_proj(len(proj_fill))
        for e in prev["at"]:
            e()
        prev["norm"]()
        normed[prev["ic"]] += 1
        if normed[prev["ic"]] == OC:
            pending_ops.extend(op_emitters(prev["ic"]))
        for e in pending_ops:
            e()
        sc2stack.close()


def build(T=2048):
    nc = bass.Bass()
    with tile.TileContext(nc) as tc:
        with ExitStack() as ctx:
            _emit(nc, tc, ctx, T)
    return nc


def make_bm():
    # bm[j, d, c]: Schraudolph add-constant for diagonal band d; dead
    # (causally masked) elements get B - MASK_SUB -> weight ~2^-109
    j = np.arange(P)[:, None, None]
    d = np.arange(4)[None, :, None]
    c = np.arange(512)[None, None, :]
    live = c >= d * 128 + j
    return np.where(live, EXP_B, EXP_B - MASK_SUB).astype(np.int16)


def make_in_maps(x, wq, bq, wk, bk, wv, bv, wo):
    bf = ml_dtypes.bfloat16
    f8 = ml_dtypes.float8_e4m3fn
    bm = make_bm()
    in_maps = []
    for c in range(8):
        b, g = c // 2, c % 2
        sl = slice(g * CL, (g + 1) * CL)
        xt = x[b].T  # [C, T]

        def fcp(a, dt):  # [C, N] -> [P, FC, N] (partition-major packing)
            n = a.shape[1]
            return np.ascontiguousarray(
                a.reshape(FC, P, n).transpose(1, 0, 2)
            ).astype(dt)

        in_maps.append(
            {
                "xT": fcp(xt, bf),
                "xT8": fcp(xt, f8),
                "wq8": fcp(wq[:, sl] * 32.0, f8),
                "wk8": fcp(wk[:, sl] * 32.0, f8),
                "wv": fcp(wv[:, sl], bf),
                "wo": np.ascontiguousarray(
                    wo[sl, :].reshape(OC, P, C).transpose(1, 0, 2)
                ).astype(bf),
                "bq": np.ascontiguousarray(bq[sl]),
                "bk": np.ascontiguousarray(bk[sl]),
                "bm": bm,
            }
        )
    return in_maps


_cache = {}


def _split_multi_waits(bir_json: bytes) -> bytes:
    """Split instructions carrying >1 sync waits into single-wait NoOp
    chains on the same engine queue.  The TPB instruction encoding has one
    wait slot; this walrus build refuses multi-wait instructions instead
    of splitting them itself."""
    import orjson

    m = orjson.loads(bir_json)
    n = 0
    for fn in m.get("functions", []):
        for blk in fn.get("blocks", []):
            out = []
            for inst in blk.get("instructions", []):
                si = inst.get("sync_info")
                waits = si.get("on_wait") if si else None
                if waits and len(waits) > 1:
                    for w in waits[:-1]:
                        n += 1
                        out.append(
                            {
                                "debug": inst.get("debug", {}),
                                "engine": inst["engine"],
                                "ins": [],
                                "outs": [],
                                "name": f"{inst['name']}_sw{n}",
                                "opcode": "NoOp",
                                "text_hint": "split_wait",
                                "sync_info": {"on_wait": [w], "on_update": []},
                            }
                        )
                    si["on_wait"] = [waits[-1]]
                out.append(inst)
            blk["instructions"] = out
    return orjson.dumps(m)


def _install_compile_patch():
    import concourse.bass_utils as bu

    if getattr(bu, "_split_waits_patched", False):
        return
    orig = bu.compile_bir_kernel

    def patched(bir_json, tmpdir, neff_name="file.neff"):
        return orig(_split_multi_waits(bir_json), tmpdir, neff_name)

    bu.compile_bir_kernel = patched
    bu._split_waits_patched = True
    try:
        import concourse.bass2jax as b2j

        b2j.compile_bir_kernel = patched
    except ImportError:
        pass


def kernel(x, wq, bq, wk, bk, wv, bv, wo, bo):
    from concourse.bass_utils import run_bass_kernel_spmd

    _install_compile_patch()

    x = np.asarray(x, np.float32)
    args = [np.asarray(a, np.float32) for a in (wq, bq, wk, bk, wv, bv, wo, bo)]
    wq, bq, wk, bk, wv, bv, wo, bo = args
    B, T, _ = x.shape

    if "nc" not in _cache:
        _cache["nc"] = build(T)
    nc = _cache["nc"]

    in_maps = make_in_maps(x, wq, bq, wk, bk, wv, bv, wo)
    res = run_bass_kernel_spmd(nc, in_maps, core_ids=list(range(8)))
    # bv is not added on-device: attn(v + bv) = attn(v) + bv, so it folds
    # into the output bias as bv @ wo
    bo_eff = bo + bv @ wo
    out = np.empty((B, T, C), np.float32)
    for b in range(B):
        out[b] = res.results[2 * b]["outp"] + res.results[2 * b + 1]["outp"] + bo_eff
    return out
